# revision 13
# baseline (speedup 1.0000x reference)
"""Trainium2 Bass kernel for nn_CACISLoss_78761110274122.

Strategy (pure data parallel, 8 cores x 64 batches):
  Build (per batch b):  eps_b = offdiag_mean(C_b);  T_ij = f_i + f_j + C_ij
                M'_ij = exp((Tlow_b - T_ij)/eps_b + ES)  (Tlow_b = 2*min f_i)
                stored transposed (M'^T rows = M' columns) in DRAM f16.
  Frank-Wolfe via cached multi-step rounds: the FW trajectory revisits a
  tiny vertex set per batch (<=8 distinct over 50 iters), so instead of one
  indirect-DMA column gather per iteration (~2.7us fixed SWDGE+DMA latency),
  keep a 4-slot SBUF column cache per batch. Each round: one bulk gather
  refills a rotating victim slot (missed vertices, or a re-fetch to keep the
  slot warm); G steps then run entirely from SBUF:
      idx=argmax(u); hit slots matched by vertex id; per-batch weight
      w=(t+1)*live; u -= w*col_slot; per-slot weight accum; t += live.
  Batches whose vertex is uncached freeze (u unchanged) until their column
  lands, then resume; per-batch step counters t_b make the lockstep exact.
  alpha is reconstructed from per-slot weight accumulators flushed on slot
  eviction:  A = sum_s Wslot_s * onehot(svid_s).
  Final:  val_b = -(A.u)/W_b^2 = alpha^T M' alpha,  W_b = t_b(t_b+1)/2.
  Loss finish on host in f64: -eps*(log val - ES) + 2*fmin - f_y.
"""

import os
from contextlib import ExitStack

import numpy as np

import concourse.bacc as bacc
import concourse.bass as bass
import concourse.tile as tile
from concourse import mybir
from concourse.bass_utils import run_bass_kernel_spmd
from concourse.masks import make_identity

B, K = 512, 256
NCORES = 8
BL = B // NCORES  # 64 batches per core
N_STEPS = 50
F32 = mybir.dt.float32
F16 = mybir.dt.float16
U32 = mybir.dt.uint32
EXP_SHIFT = 1.0
NSLOT = 3
FW_G = int(os.environ.get("KM_G", "3"))  # steps per round
FW_T = int(os.environ.get("KM_T", "48"))  # total step slots
INVALID_U = 65535  # slot-id sentinel, never equals a vertex id (< 256)
ALU = mybir.AluOpType
AFT = mybir.ActivationFunctionType
AXL = mybir.AxisListType


def _build_mt(tc, ctx, C_l, scores_l, singles, eps_row, fpack, mt_dram, u):
    """Build M'^T into mt_dram (f16) + row sums -> u (f16), eps/fmin tiles.

    Identical structure to the tuned baseline build: group-pipelined
    C load -> T build (+f_i) -> transpose -> exp(scale,bias) -> row sums.
    """
    nc = tc.nc
    ct_pool = ctx.enter_context(tc.tile_pool(name="ct", bufs=6))
    mt_pool = ctx.enter_context(tc.tile_pool(name="mt", bufs=3))
    eps_pool = ctx.enter_context(tc.tile_pool(name="eps", bufs=2))
    ps_small = ctx.enter_context(tc.tile_pool(name="psS", bufs=2, space="PSUM"))
    ps_tt = ctx.enter_context(tc.tile_pool(name="psTT", bufs=4, space="PSUM"))
    ps_r0 = ctx.enter_context(tc.tile_pool(name="psR0", bufs=1, space="PSUM"))

    ident = singles.tile([128, 128], F32)
    make_identity(nc, ident[:])
    ones_col = singles.tile([128, 1], F32)
    nc.vector.memset(ones_col[:], 1.0)
    ones_col_h = singles.tile([128, 1], F16)
    nc.vector.memset(ones_col_h[:], 1.0)
    ones_row = singles.tile([1, 128], F32)
    nc.vector.memset(ones_row[:], 1.0)

    # ---- scores -> f = scores/2, reductions, row/col layouts ----
    scores_sb = singles.tile([BL, K], F32)
    nc.sync.dma_start(out=scores_sb[:], in_=scores_l[:, :])
    fhalf = singles.tile([BL, K], F32)
    nc.vector.tensor_scalar_mul(fhalf[:], scores_sb[:], 0.5)
    nc.vector.reduce_sum(out=fpack[:, 0:1], in_=fhalf[:], axis=AXL.X)
    nc.vector.tensor_reduce(out=fpack[:, 1:2], in_=fhalf[:], axis=AXL.X, op=ALU.min)
    fT_ps = ps_small.tile([128, 2 * BL], F32, tag="small")
    for ib in range(2):
        nc.tensor.transpose(
            out=fT_ps[:, ib * BL : (ib + 1) * BL],
            in_=fhalf[:, ib * 128 : (ib + 1) * 128],
            identity=ident[0:BL, 0:BL],
        )
    fT_sb = singles.tile([128, 2 * BL], F32)
    nc.vector.tensor_copy(out=fT_sb[:], in_=fT_ps[:])

    GRP = 8
    NG = BL // GRP
    collector = singles.tile([128, 4 * BL], F32)

    fpT_ps = ps_small.tile([1, 2 * BL], F32, tag="small")
    for c in range(2):
        nc.tensor.transpose(
            out=fpT_ps[:, c * BL : (c + 1) * BL],
            in_=fpack[:, c : c + 1],
            identity=ident[0:BL, 0:BL],
        )
    frows = singles.tile([1, 2 * BL], F32)
    nc.vector.tensor_copy(out=frows[:], in_=fpT_ps[:])

    r0c = ps_r0.tile([128, K], F32)
    scb = singles.tile([128, 16 * NG], F32)
    biasv = singles.tile([128, 2 * BL], F32)
    coll2 = collector[:].rearrange("p (s c) -> p s c", s=2)

    for g in range(NG):
        ct = ct_pool.tile([128, 2 * GRP, K], F32, tag="ct")
        src_ap = bass.AP(
            tensor=C_l.tensor,
            offset=g * GRP * K * K,
            ap=[[K, 128], [128 * K, 2 * GRP], [1, K]],
        )
        nc.sync.dma_start(out=ct[:], in_=src_ap)
        for ih in range(2):
            diag_ap = bass.AP(
                tensor=C_l.tensor,
                offset=g * GRP * K * K + ih * (K + 1) * 128,
                ap=[[K + 1, 128], [K * K, GRP]],
            )
            c0 = 2 * BL + g * 16 + ih * 8
            nc.scalar.dma_start(out=collector[:, c0 : c0 + 8], in_=diag_ap)
        for b2 in range(GRP):
            b = g * GRP + b2
            for ib in range(2):
                c0 = g * 16 + ib * 8 + b2
                nc.vector.tensor_scalar(
                    out=ct[:, b2 * 2 + ib, :],
                    in0=ct[:, b2 * 2 + ib, :],
                    scalar1=fT_sb[:, ib * BL + b : ib * BL + b + 1],
                    scalar2=0.0,
                    op0=ALU.add,
                    op1=ALU.add,
                    accum_out=collector[:, c0 : c0 + 1],
                )

        # eps chain for this group
        gs = slice(g * 8, (g + 1) * 8)
        colsum_ps = ps_small.tile([1, 32], F32, tag="small")
        nc.tensor.matmul(
            out=colsum_ps[:],
            lhsT=ones_col[:],
            rhs=coll2[:, :, g * 16 : g * 16 + 16],
            start=True,
            stop=True,
        )
        srow = eps_pool.tile([1, 32], F32, tag="srow")
        nc.vector.tensor_copy(out=srow[:], in_=colsum_ps[:])
        sc = eps_pool.tile([1, 8], F32, tag="sc")
        nc.vector.tensor_add(out=sc[:], in0=srow[0:1, 0:8], in1=srow[0:1, 8:16])
        nc.vector.scalar_tensor_tensor(
            out=sc[:], in0=frows[0:1, gs], scalar=-1.0 * K, in1=sc[:],
            op0=ALU.mult, op1=ALU.add,
        )
        tr = eps_pool.tile([1, 8], F32, tag="tr")
        nc.vector.tensor_add(out=tr[:], in0=srow[0:1, 16:24], in1=srow[0:1, 24:32])
        nc.vector.tensor_sub(out=sc[:], in0=sc[:], in1=tr[:])
        nc.vector.tensor_scalar(
            out=eps_row[0:1, gs], in0=sc[:], scalar1=1.0 / (K * K - K),
            scalar2=1e-8, op0=ALU.mult, op1=ALU.max,
        )
        rec = eps_pool.tile([1, 8], F32, tag="rec")
        nc.vector.reciprocal(out=rec[:], in_=eps_row[0:1, gs])
        sr = eps_pool.tile([1, 8], F32, tag="sr")
        nc.vector.tensor_scalar_mul(sr[:], rec[:], -1.0)
        br = eps_pool.tile([1, 8], F32, tag="br")
        nc.vector.scalar_tensor_tensor(
            out=br[:], in0=frows[0:1, BL + g * 8 : BL + (g + 1) * 8],
            scalar=-2.0, in1=sr[:], op0=ALU.mult, op1=ALU.mult,
        )
        nc.vector.tensor_scalar_add(br[:], br[:], EXP_SHIFT)
        scb_ps = ps_small.tile([128, 16], F32, tag="small")
        nc.tensor.matmul(
            out=scb_ps[:, 0:8], lhsT=ones_row[:, :], rhs=sr[:], start=True, stop=True
        )
        nc.tensor.matmul(
            out=scb_ps[:, 8:16], lhsT=ones_row[:, :], rhs=br[:], start=True, stop=True
        )
        nc.vector.tensor_copy(out=scb[:, g * 16 : (g + 1) * 16], in_=scb_ps[:])
        for jb in range(2):
            sl = slice(jb * BL + g * 8, jb * BL + (g + 1) * 8)
            nc.vector.tensor_mul(
                out=biasv[:, sl], in0=fT_sb[:, sl], in1=scb[:, g * 16 : g * 16 + 8]
            )
            nc.vector.tensor_add(
                out=biasv[:, sl], in0=biasv[:, sl],
                in1=scb[:, g * 16 + 8 : g * 16 + 16],
            )

        # transpose -> exp -> rowsum matmuls -> M'^T store
        mt_sb = mt_pool.tile([128, 2 * GRP, K], F16, tag="mt")
        for b2 in range(GRP):
            b = g * GRP + b2
            tt_ps = ps_tt.tile([128, 2, K], F32, tag="tt")
            for jb in range(2):
                for ib in range(2):
                    nc.tensor.transpose(
                        out=tt_ps[:, jb, ib * 128 : (ib + 1) * 128],
                        in_=ct[:, b2 * 2 + ib, jb * 128 : (jb + 1) * 128],
                        identity=ident[:],
                    )
            for jb in range(2):
                m = b2 * 2 + jb
                nc.scalar.activation(
                    out=mt_sb[:, m, :],
                    in_=tt_ps[:, jb, :],
                    func=AFT.Exp,
                    bias=biasv[:, jb * BL + b : jb * BL + b + 1],
                    scale=scb[:, g * 16 + b2 : g * 16 + b2 + 1],
                )
                for ib in range(2):
                    col = jb * 128 + ib * BL + b
                    nc.tensor.matmul(
                        out=r0c[:, col : col + 1],
                        lhsT=mt_sb[:, m, ib * 128 : (ib + 1) * 128],
                        rhs=ones_col_h[:],
                        start=True,
                        stop=True,
                    )
        dst_ap = bass.AP(
            tensor=mt_dram.tensor,
            offset=g * GRP * K * K,
            ap=[[K, 128], [128 * K, 2 * GRP], [1, K]],
        )
        nc.sync.dma_start(out=dst_ap, in_=mt_sb[:])

    # rowsums -> u0 = -rowsum (f16)
    r0s = singles.tile([128, 128], F32)
    nc.vector.tensor_copy(out=r0s[:], in_=r0c[:, 0:128])
    nc.vector.tensor_add(out=r0s[:], in0=r0s[:], in1=r0c[:, 128:K])
    r0T_ps = ps_small.tile([128, 128], F32, tag="small")
    nc.tensor.transpose(out=r0T_ps[:], in_=r0s[:], identity=ident[:])
    nc.vector.tensor_scalar_mul(u[:, 0:128], r0T_ps[0:BL, :], -1.0)
    nc.vector.tensor_scalar_mul(u[:, 128:K], r0T_ps[BL : 2 * BL, :], -1.0)


def _kernel_body(tc, C_l, scores_l, val_o, eps_o, fmin_o, t_o):
    nc = tc.nc
    with ExitStack() as ctx:
        singles = ctx.enter_context(tc.tile_pool(name="singles", bufs=1))
        fw_pool = ctx.enter_context(tc.tile_pool(name="fw", bufs=3))
        dram = ctx.enter_context(tc.tile_pool(name="dram", bufs=1, space="DRAM"))

        mt_dram = dram.tile([BL * K, K], F16)
        eps_row = singles.tile([1, BL], F32)
        fpack = singles.tile([BL, 2], F32)
        u = singles.tile([BL, K], F16)

        _build_mt(tc, ctx, C_l, scores_l, singles, eps_row, fpack, mt_dram, u)

        # ---- FW state ----
        rowbase = singles.tile([BL, 1], U32)
        nc.gpsimd.iota(rowbase[:], pattern=[[0, 1]], base=0, channel_multiplier=K)
        iota_u = singles.tile([BL, K], U32)
        nc.gpsimd.iota(iota_u[:], pattern=[[1, K]], base=0, channel_multiplier=0)
        iota_h = singles.tile([BL, K], F16)
        nc.vector.tensor_copy(out=iota_h[:], in_=iota_u[:])

        cache = [singles.tile([BL, K], F16, name=f"cache{s}") for s in range(NSLOT)]
        ohslot = [singles.tile([BL, K], F16, name=f"ohslot{s}") for s in range(NSLOT)]
        for s in range(NSLOT):
            nc.vector.memset(cache[s][:], 0.0)
            nc.vector.memset(ohslot[s][:], 0.0)
        svid_f = singles.tile([BL, NSLOT], F32)
        nc.vector.memset(svid_f[:], float(INVALID_U))
        wslot = singles.tile([BL, NSLOT], F32)
        nc.vector.memset(wslot[:], 0.0)
        A = singles.tile([BL, K], F32)
        nc.vector.memset(A[:], 0.0)
        tcnt = singles.tile([BL, 1], F32)
        nc.gpsimd.memset(tcnt[:], 0.0)
        flt = singles.tile([BL, K], F32)  # flush scratch (Pool)

        n_rounds = (FW_T + FW_G - 1) // FW_G + 1
        step_no = 0

        # initial argmax of u0
        vals8 = fw_pool.tile([BL, 8], F16, tag="vals8")
        idx8 = fw_pool.tile([BL, 8], U32, tag="idx8")
        nc.vector.max(out=vals8[:], in_=u[:])
        nc.vector.max_index(out=idx8[:], in_max=vals8[:], in_values=u[:])
        amin = fw_pool.tile([BL, 1], F32, tag="amin")
        nc.vector.memset(amin[:], 0.0)  # "missed" before round 1

        pending = None
        for r in range(n_rounds):
            v = r % NSLOT
            # ---- boundary ----
            # land the previous round's gather (svid update + slot onehot)
            if pending is not None:
                pv, plnd_f = pending
                nc.gpsimd.tensor_copy(out=svid_f[:, pv : pv + 1], in_=plnd_f[:])
                nc.gpsimd.tensor_scalar(
                    out=ohslot[pv][:], in0=iota_h[:], scalar1=plnd_f[:, 0:1],
                    scalar2=0.0, op0=ALU.is_equal, op1=ALU.add,
                )
            # flush victim's A contribution (Pool: mult then add)
            nc.gpsimd.tensor_scalar(
                out=flt[:], in0=ohslot[v][:], scalar1=wslot[:, v : v + 1],
                scalar2=0.0, op0=ALU.mult, op1=ALU.add,
            )
            nc.gpsimd.tensor_add(out=A[:], in0=A[:], in1=flt[:])
            nc.gpsimd.memset(wslot[:, v : v + 1], 0.0)

            # gather idx selection (all on Pool; DVE only runs the steps)
            idx_f = fw_pool.tile([BL, 1], F32, tag="idxf")
            nc.gpsimd.tensor_copy(out=idx_f[:], in_=idx8[:, 0:1])
            mnow = fw_pool.tile([BL, NSLOT], F32, tag="mnow")
            nc.gpsimd.tensor_scalar(
                out=mnow[:], in0=svid_f[:], scalar1=idx_f[:, 0:1], scalar2=0.0,
                op0=ALU.is_equal, op1=ALU.add,
            )
            hitn = fw_pool.tile([BL, 1], F32, tag="hitn")
            nc.gpsimd.tensor_add(out=hitn[:], in0=mnow[:, 0:1], in1=mnow[:, 1:2])
            nc.gpsimd.tensor_add(out=hitn[:], in0=hitn[:], in1=mnow[:, 2:3])
            inval = fw_pool.tile([BL, 1], F32, tag="inval")
            nc.gpsimd.tensor_scalar(
                out=inval[:], in0=svid_f[:, v : v + 1], scalar1=float(INVALID_U),
                scalar2=0.0, op0=ALU.is_equal, op1=ALU.add,
            )
            # gidx = idx + hit*(1-inval)*(svid_v - idx)
            hi = fw_pool.tile([BL, 1], F32, tag="hi")
            nc.gpsimd.tensor_scalar(
                out=hi[:], in0=inval[:], scalar1=-1.0, scalar2=1.0,
                op0=ALU.mult, op1=ALU.add,
            )
            nc.gpsimd.tensor_mul(out=hi[:], in0=hi[:], in1=hitn[:])
            gidx_f = fw_pool.tile([BL, 1], F32, tag="gidxf")
            nc.gpsimd.tensor_sub(out=gidx_f[:], in0=svid_f[:, v : v + 1], in1=idx_f[:])
            nc.gpsimd.tensor_mul(out=gidx_f[:], in0=gidx_f[:], in1=hi[:])
            nc.gpsimd.tensor_add(out=gidx_f[:], in0=gidx_f[:], in1=idx_f[:])
            # landing id: INVALID when hit & victim-was-invalid (duplicate guard)
            dup = fw_pool.tile([BL, 1], F32, tag="dup")
            nc.gpsimd.tensor_mul(out=dup[:], in0=hitn[:], in1=inval[:])
            lnd_f = fw_pool.tile([BL, 1], F32, tag="lndf")
            nc.gpsimd.tensor_scalar(
                out=lnd_f[:], in0=gidx_f[:], scalar1=-1.0, scalar2=float(INVALID_U),
                op0=ALU.mult, op1=ALU.add,
            )
            nc.gpsimd.tensor_mul(out=lnd_f[:], in0=lnd_f[:], in1=dup[:])
            nc.gpsimd.tensor_add(out=lnd_f[:], in0=lnd_f[:], in1=gidx_f[:])
            gidx_u = fw_pool.tile([BL, 1], U32, tag="gidxu")
            nc.gpsimd.tensor_copy(out=gidx_u[:], in_=gidx_f[:])
            # invalidate victim for this round (after gidx computed)
            nc.gpsimd.memset(svid_f[:, v : v + 1], float(INVALID_U))

            idxg = fw_pool.tile([BL, 1], U32, tag="idxg")
            nc.gpsimd.tensor_add(out=idxg[:], in0=gidx_u[:], in1=rowbase[:])
            nc.gpsimd.indirect_dma_start(
                out=cache[v][:],
                out_offset=None,
                in_=mt_dram[:],
                in_offset=bass.IndirectOffsetOnAxis(ap=idxg[:, 0:1], axis=0),
            )
            pending = (v, lnd_f)

            if r == 0:
                continue

            # ---- G steps from cache (victim slot v excluded) ----
            slots = [s for s in range(NSLOT) if s != v]
            for g in range(FW_G):
                if step_no >= FW_T:
                    break
                step_no += 1
                # wneg = -(t+1) ; a_s = (svid_s == idx) * wneg
                wneg = fw_pool.tile([BL, 1], F32, tag="wneg")
                nc.gpsimd.tensor_scalar(
                    out=wneg[:], in0=tcnt[:], scalar1=1.0, scalar2=-1.0,
                    op0=ALU.add, op1=ALU.mult,
                )
                idx_fs = fw_pool.tile([BL, 1], F32, tag="idxfs")
                nc.vector.tensor_copy(out=idx_fs[:], in_=idx8[:, 0:1])
                a = fw_pool.tile([BL, NSLOT], F32, tag="a")
                nc.vector.tensor_scalar(
                    out=a[:], in0=svid_f[:], scalar1=idx_fs[:, 0:1],
                    scalar2=wneg[:, 0:1], op0=ALU.is_equal, op1=ALU.mult,
                )
                amin = fw_pool.tile([BL, 1], F32, tag="amin")
                nc.vector.tensor_reduce(out=amin[:], in_=a[:], axis=AXL.X, op=ALU.min)
                # Pool bookkeeping: live, t += live, wslot -= a
                live = fw_pool.tile([BL, 1], F32, tag="live")
                nc.gpsimd.tensor_scalar(
                    out=live[:], in0=amin[:], scalar1=-0.5, scalar2=0.0,
                    op0=ALU.is_lt, op1=ALU.add,
                )
                nc.gpsimd.tensor_add(out=tcnt[:], in0=tcnt[:], in1=live[:])
                nc.gpsimd.tensor_sub(out=wslot[:], in0=wslot[:], in1=a[:])
                # u update over the two non-victim slots
                if step_no == 1:
                    s0, s1 = slots
                    nc.vector.tensor_scalar(
                        out=u[:], in0=cache[s0][:], scalar1=a[:, s0 : s0 + 1],
                        scalar2=0.0, op0=ALU.mult, op1=ALU.add,
                    )
                    nc.vector.scalar_tensor_tensor(
                        out=u[:], in0=cache[s1][:], scalar=a[:, s1 : s1 + 1],
                        in1=u[:], op0=ALU.mult, op1=ALU.add,
                    )
                else:
                    for s in slots:
                        nc.vector.scalar_tensor_tensor(
                            out=u[:], in0=cache[s][:], scalar=a[:, s : s + 1],
                            in1=u[:], op0=ALU.mult, op1=ALU.add,
                        )
                vals8 = fw_pool.tile([BL, 8], F16, tag="vals8")
                idx8 = fw_pool.tile([BL, 8], U32, tag="idx8")
                nc.vector.max(out=vals8[:], in_=u[:])
                nc.vector.max_index(out=idx8[:], in_max=vals8[:], in_values=u[:])

        # ---- final flush of all slots + val ----
        for s in range(NSLOT):
            nc.vector.scalar_tensor_tensor(
                out=A[:], in0=ohslot[s][:], scalar=wslot[:, s : s + 1],
                in1=A[:], op0=ALU.mult, op1=ALU.add,
            )
        junk = singles.tile([BL, K], F32)
        val_sb = singles.tile([BL, 1], F32)
        nc.vector.tensor_mul(out=junk[:], in0=A[:], in1=u[:])
        nc.vector.reduce_sum(out=val_sb[:], in_=junk[:], axis=AXL.X)
        nc.sync.dma_start(out=val_o[:, :], in_=val_sb[:])
        nc.sync.dma_start(out=eps_o[:, :], in_=eps_row[:])
        nc.sync.dma_start(out=fmin_o[:, :], in_=fpack[:, 1:2])
        nc.sync.dma_start(out=t_o[:, :], in_=tcnt[:])


_NC = None


def _get_nc():
    global _NC
    if _NC is None:
        nc = bacc.Bacc(
            "TRN2",
            target_bir_lowering=False,
            debug=False,
            enable_asserts=False,
            num_devices=NCORES,
        )
        C_l = nc.dram_tensor("C_l", (BL, K, K), F32, kind="ExternalInput").ap()
        scores_l = nc.dram_tensor("scores_l", (BL, K), F32, kind="ExternalInput").ap()
        val_o = nc.dram_tensor("val_o", (BL, 1), F32, kind="ExternalOutput").ap()
        eps_o = nc.dram_tensor("eps_o", (1, BL), F32, kind="ExternalOutput").ap()
        fmin_o = nc.dram_tensor("fmin_o", (BL, 1), F32, kind="ExternalOutput").ap()
        t_o = nc.dram_tensor("t_o", (BL, 1), F32, kind="ExternalOutput").ap()
        with tile.TileContext(nc) as tc:
            _kernel_body(tc, C_l, scores_l, val_o, eps_o, fmin_o, t_o)
        nc.compile()
        _NC = nc
    return _NC


def _finish(results, scores, targets):
    vals = np.concatenate([r["val_o"][:, 0] for r in results]).astype(np.float64)
    eps = np.concatenate([r["eps_o"][0, :] for r in results]).astype(np.float64)
    fmin = np.concatenate([r["fmin_o"][:, 0] for r in results]).astype(np.float64)
    t = np.concatenate([r["t_o"][:, 0] for r in results]).astype(np.float64)
    W = np.maximum(t * (t + 1.0) / 2.0, 1.0)
    val = np.maximum(-vals / (W * W), 1e-300)
    f_y = scores[np.arange(B), targets].astype(np.float64)
    loss = -eps * (np.log(val) - EXP_SHIFT) + 2.0 * fmin - f_y
    return np.float32(loss.mean())


def _run(inputs, **spmd_kwargs):
    scores = np.ascontiguousarray(np.asarray(inputs["scores"], dtype=np.float32))
    targets = np.asarray(inputs["targets"]).astype(np.int64)
    C = np.asarray(inputs["C"], dtype=np.float32)
    nc = _get_nc()
    in_maps = []
    for c in range(NCORES):
        sl = slice(c * BL, (c + 1) * BL)
        in_maps.append(
            {
                "C_l": np.ascontiguousarray(C[sl]),
                "scores_l": np.ascontiguousarray(scores[sl]),
            }
        )
    res = run_bass_kernel_spmd(nc, in_maps, core_ids=list(range(NCORES)), **spmd_kwargs)
    return _finish(res.results, scores, targets), res


def kernel(**inputs) -> np.ndarray:
    out, _ = _run(inputs)
    return out


# revision 14
# speedup vs baseline: 1.0471x; 1.0471x over previous
"""Trainium2 Bass kernel for nn_CACISLoss_78761110274122.

Strategy (pure data parallel, 8 cores x 64 batches):
  Build (per batch b):  eps_b = offdiag_mean(C_b);  T_ij = f_i + f_j + C_ij
                M'_ij = exp((Tlow_b - T_ij)/eps_b + ES)  (Tlow_b = 2*min f_i)
                stored transposed (M'^T rows = M' columns) in DRAM f16.
  Frank-Wolfe via cached multi-step rounds: the FW trajectory revisits a
  tiny vertex set per batch (<=8 distinct over 50 iters), so instead of one
  indirect-DMA column gather per iteration (~2.7us fixed SWDGE+DMA latency),
  keep a 4-slot SBUF column cache per batch. Each round: one bulk gather
  refills a rotating victim slot (missed vertices, or a re-fetch to keep the
  slot warm); G steps then run entirely from SBUF:
      idx=argmax(u); hit slots matched by vertex id; per-batch weight
      w=(t+1)*live; u -= w*col_slot; per-slot weight accum; t += live.
  Batches whose vertex is uncached freeze (u unchanged) until their column
  lands, then resume; per-batch step counters t_b make the lockstep exact.
  alpha is reconstructed from per-slot weight accumulators flushed on slot
  eviction:  A = sum_s Wslot_s * onehot(svid_s).
  Final:  val_b = -(A.u)/W_b^2 = alpha^T M' alpha,  W_b = t_b(t_b+1)/2.
  Loss finish on host in f64: -eps*(log val - ES) + 2*fmin - f_y.
"""

import os
from contextlib import ExitStack

import numpy as np

import concourse.bacc as bacc
import concourse.bass as bass
import concourse.tile as tile
from concourse import mybir
from concourse.bass_utils import run_bass_kernel_spmd
from concourse.masks import make_identity

B, K = 512, 256
NCORES = 8
BL = B // NCORES  # 64 batches per core
N_STEPS = 50
F32 = mybir.dt.float32
F16 = mybir.dt.float16
U32 = mybir.dt.uint32
EXP_SHIFT = 1.0
NSLOT = 3
FW_G = int(os.environ.get("KM_G", "3"))  # steps per round
FW_T = int(os.environ.get("KM_T", "48"))  # total step slots
INVALID_U = 65535  # slot-id sentinel, never equals a vertex id (< 256)
ALU = mybir.AluOpType
AFT = mybir.ActivationFunctionType
AXL = mybir.AxisListType


def _build_mt(tc, ctx, C_l, scores_l, singles, eps_row, fpack, mt_dram, u):
    """Build M'^T into mt_dram (f16) + row sums -> u (f16), eps/fmin tiles.

    Identical structure to the tuned baseline build: group-pipelined
    C load -> T build (+f_i) -> transpose -> exp(scale,bias) -> row sums.
    """
    nc = tc.nc
    ct_pool = ctx.enter_context(tc.tile_pool(name="ct", bufs=6))
    mt_pool = ctx.enter_context(tc.tile_pool(name="mt", bufs=3))
    eps_pool = ctx.enter_context(tc.tile_pool(name="eps", bufs=2))
    ps_small = ctx.enter_context(tc.tile_pool(name="psS", bufs=2, space="PSUM"))
    ps_tt = ctx.enter_context(tc.tile_pool(name="psTT", bufs=4, space="PSUM"))
    ps_r0 = ctx.enter_context(tc.tile_pool(name="psR0", bufs=1, space="PSUM"))

    ident = singles.tile([128, 128], F32)
    make_identity(nc, ident[:])
    ones_col = singles.tile([128, 1], F32)
    nc.vector.memset(ones_col[:], 1.0)
    ones_col_h = singles.tile([128, 1], F16)
    nc.vector.memset(ones_col_h[:], 1.0)
    ones_row = singles.tile([1, 128], F32)
    nc.vector.memset(ones_row[:], 1.0)

    # ---- scores -> f = scores/2, reductions, row/col layouts ----
    scores_sb = singles.tile([BL, K], F32)
    nc.sync.dma_start(out=scores_sb[:], in_=scores_l[:, :])
    fhalf = singles.tile([BL, K], F32)
    nc.vector.tensor_scalar_mul(fhalf[:], scores_sb[:], 0.5)
    nc.vector.reduce_sum(out=fpack[:, 0:1], in_=fhalf[:], axis=AXL.X)
    nc.vector.tensor_reduce(out=fpack[:, 1:2], in_=fhalf[:], axis=AXL.X, op=ALU.min)
    fT_ps = ps_small.tile([128, 2 * BL], F32, tag="small")
    for ib in range(2):
        nc.tensor.transpose(
            out=fT_ps[:, ib * BL : (ib + 1) * BL],
            in_=fhalf[:, ib * 128 : (ib + 1) * 128],
            identity=ident[0:BL, 0:BL],
        )
    fT_sb = singles.tile([128, 2 * BL], F32)
    nc.vector.tensor_copy(out=fT_sb[:], in_=fT_ps[:])

    GRP = 8
    NG = BL // GRP
    collector = singles.tile([128, 4 * BL], F32)

    fpT_ps = ps_small.tile([1, 2 * BL], F32, tag="small")
    for c in range(2):
        nc.tensor.transpose(
            out=fpT_ps[:, c * BL : (c + 1) * BL],
            in_=fpack[:, c : c + 1],
            identity=ident[0:BL, 0:BL],
        )
    frows = singles.tile([1, 2 * BL], F32)
    nc.vector.tensor_copy(out=frows[:], in_=fpT_ps[:])

    r0c = ps_r0.tile([128, K], F32)
    scb = singles.tile([128, 16 * NG], F32)
    biasv = singles.tile([128, 2 * BL], F32)
    coll2 = collector[:].rearrange("p (s c) -> p s c", s=2)

    for g in range(NG):
        ct = ct_pool.tile([128, 2 * GRP, K], F32, tag="ct")
        src_ap = bass.AP(
            tensor=C_l.tensor,
            offset=g * GRP * K * K,
            ap=[[K, 128], [128 * K, 2 * GRP], [1, K]],
        )
        nc.sync.dma_start(out=ct[:], in_=src_ap)
        for ih in range(2):
            diag_ap = bass.AP(
                tensor=C_l.tensor,
                offset=g * GRP * K * K + ih * (K + 1) * 128,
                ap=[[K + 1, 128], [K * K, GRP]],
            )
            c0 = 2 * BL + g * 16 + ih * 8
            nc.scalar.dma_start(out=collector[:, c0 : c0 + 8], in_=diag_ap)
        for b2 in range(GRP):
            b = g * GRP + b2
            for ib in range(2):
                c0 = g * 16 + ib * 8 + b2
                nc.vector.tensor_scalar(
                    out=ct[:, b2 * 2 + ib, :],
                    in0=ct[:, b2 * 2 + ib, :],
                    scalar1=fT_sb[:, ib * BL + b : ib * BL + b + 1],
                    scalar2=0.0,
                    op0=ALU.add,
                    op1=ALU.add,
                    accum_out=collector[:, c0 : c0 + 1],
                )

        # eps chain for this group
        gs = slice(g * 8, (g + 1) * 8)
        colsum_ps = ps_small.tile([1, 32], F32, tag="small")
        nc.tensor.matmul(
            out=colsum_ps[:],
            lhsT=ones_col[:],
            rhs=coll2[:, :, g * 16 : g * 16 + 16],
            start=True,
            stop=True,
        )
        srow = eps_pool.tile([1, 32], F32, tag="srow")
        nc.vector.tensor_copy(out=srow[:], in_=colsum_ps[:])
        sc = eps_pool.tile([1, 8], F32, tag="sc")
        nc.vector.tensor_add(out=sc[:], in0=srow[0:1, 0:8], in1=srow[0:1, 8:16])
        nc.vector.scalar_tensor_tensor(
            out=sc[:], in0=frows[0:1, gs], scalar=-1.0 * K, in1=sc[:],
            op0=ALU.mult, op1=ALU.add,
        )
        tr = eps_pool.tile([1, 8], F32, tag="tr")
        nc.vector.tensor_add(out=tr[:], in0=srow[0:1, 16:24], in1=srow[0:1, 24:32])
        nc.vector.tensor_sub(out=sc[:], in0=sc[:], in1=tr[:])
        nc.vector.tensor_scalar(
            out=eps_row[0:1, gs], in0=sc[:], scalar1=1.0 / (K * K - K),
            scalar2=1e-8, op0=ALU.mult, op1=ALU.max,
        )
        rec = eps_pool.tile([1, 8], F32, tag="rec")
        nc.vector.reciprocal(out=rec[:], in_=eps_row[0:1, gs])
        sr = eps_pool.tile([1, 8], F32, tag="sr")
        nc.vector.tensor_scalar_mul(sr[:], rec[:], -1.0)
        br = eps_pool.tile([1, 8], F32, tag="br")
        nc.vector.scalar_tensor_tensor(
            out=br[:], in0=frows[0:1, BL + g * 8 : BL + (g + 1) * 8],
            scalar=-2.0, in1=sr[:], op0=ALU.mult, op1=ALU.mult,
        )
        nc.vector.tensor_scalar_add(br[:], br[:], EXP_SHIFT)
        scb_ps = ps_small.tile([128, 16], F32, tag="small")
        nc.tensor.matmul(
            out=scb_ps[:, 0:8], lhsT=ones_row[:, :], rhs=sr[:], start=True, stop=True
        )
        nc.tensor.matmul(
            out=scb_ps[:, 8:16], lhsT=ones_row[:, :], rhs=br[:], start=True, stop=True
        )
        nc.vector.tensor_copy(out=scb[:, g * 16 : (g + 1) * 16], in_=scb_ps[:])
        for jb in range(2):
            sl = slice(jb * BL + g * 8, jb * BL + (g + 1) * 8)
            nc.vector.tensor_mul(
                out=biasv[:, sl], in0=fT_sb[:, sl], in1=scb[:, g * 16 : g * 16 + 8]
            )
            nc.vector.tensor_add(
                out=biasv[:, sl], in0=biasv[:, sl],
                in1=scb[:, g * 16 + 8 : g * 16 + 16],
            )

        # transpose -> exp -> rowsum matmuls -> M'^T store
        mt_sb = mt_pool.tile([128, 2 * GRP, K], F16, tag="mt")
        for b2 in range(GRP):
            b = g * GRP + b2
            tt_ps = ps_tt.tile([128, 2, K], F32, tag="tt")
            for jb in range(2):
                for ib in range(2):
                    nc.tensor.transpose(
                        out=tt_ps[:, jb, ib * 128 : (ib + 1) * 128],
                        in_=ct[:, b2 * 2 + ib, jb * 128 : (jb + 1) * 128],
                        identity=ident[:],
                    )
            for jb in range(2):
                m = b2 * 2 + jb
                nc.scalar.activation(
                    out=mt_sb[:, m, :],
                    in_=tt_ps[:, jb, :],
                    func=AFT.Exp,
                    bias=biasv[:, jb * BL + b : jb * BL + b + 1],
                    scale=scb[:, g * 16 + b2 : g * 16 + b2 + 1],
                )
                for ib in range(2):
                    col = jb * 128 + ib * BL + b
                    nc.tensor.matmul(
                        out=r0c[:, col : col + 1],
                        lhsT=mt_sb[:, m, ib * 128 : (ib + 1) * 128],
                        rhs=ones_col_h[:],
                        start=True,
                        stop=True,
                    )
        dst_ap = bass.AP(
            tensor=mt_dram.tensor,
            offset=g * GRP * K * K,
            ap=[[K, 128], [128 * K, 2 * GRP], [1, K]],
        )
        nc.sync.dma_start(out=dst_ap, in_=mt_sb[:])

    # rowsums -> u0 = -rowsum (f16)
    r0s = singles.tile([128, 128], F32)
    nc.vector.tensor_copy(out=r0s[:], in_=r0c[:, 0:128])
    nc.vector.tensor_add(out=r0s[:], in0=r0s[:], in1=r0c[:, 128:K])
    r0T_ps = ps_small.tile([128, 128], F32, tag="small")
    nc.tensor.transpose(out=r0T_ps[:], in_=r0s[:], identity=ident[:])
    nc.vector.tensor_scalar_mul(u[:, 0:128], r0T_ps[0:BL, :], -1.0)
    nc.vector.tensor_scalar_mul(u[:, 128:K], r0T_ps[BL : 2 * BL, :], -1.0)


def _kernel_body(tc, C_l, scores_l, val_o, eps_o, fmin_o, t_o):
    nc = tc.nc
    with ExitStack() as ctx:
        singles = ctx.enter_context(tc.tile_pool(name="singles", bufs=1))
        fw_pool = ctx.enter_context(tc.tile_pool(name="fw", bufs=3))
        dram = ctx.enter_context(tc.tile_pool(name="dram", bufs=1, space="DRAM"))

        mt_dram = dram.tile([BL * K, K], F16)
        eps_row = singles.tile([1, BL], F32)
        fpack = singles.tile([BL, 2], F32)
        u = singles.tile([BL, K], F16)

        _build_mt(tc, ctx, C_l, scores_l, singles, eps_row, fpack, mt_dram, u)

        # ---- FW state ----
        rowbase = singles.tile([BL, 1], U32)
        nc.gpsimd.iota(rowbase[:], pattern=[[0, 1]], base=0, channel_multiplier=K)
        iota_u = singles.tile([BL, K], U32)
        nc.gpsimd.iota(iota_u[:], pattern=[[1, K]], base=0, channel_multiplier=0)
        iota_h = singles.tile([BL, K], F16)
        nc.vector.tensor_copy(out=iota_h[:], in_=iota_u[:])

        cache = [singles.tile([BL, K], F16, name=f"cache{s}") for s in range(NSLOT)]
        ohslot = [singles.tile([BL, K], F16, name=f"ohslot{s}") for s in range(NSLOT)]
        for s in range(NSLOT):
            nc.vector.memset(cache[s][:], 0.0)
            nc.vector.memset(ohslot[s][:], 0.0)
        svid_f = singles.tile([BL, NSLOT], F32)
        nc.vector.memset(svid_f[:], float(INVALID_U))
        wslot = singles.tile([BL, NSLOT], F32)
        nc.vector.memset(wslot[:], 0.0)
        A = singles.tile([BL, K], F32)
        nc.vector.memset(A[:], 0.0)
        tcnt = singles.tile([BL, 1], F32)
        nc.gpsimd.memset(tcnt[:], 0.0)
        flt = singles.tile([BL, K], F32)  # flush scratch (Pool)

        n_rounds = (FW_T + FW_G - 1) // FW_G + 1
        step_no = 0

        # initial argmax of u0
        vals8 = fw_pool.tile([BL, 8], F16, tag="vals8")
        idx8 = fw_pool.tile([BL, 8], U32, tag="idx8")
        nc.vector.max(out=vals8[:], in_=u[:])
        nc.vector.max_index(out=idx8[:], in_max=vals8[:], in_values=u[:])
        amin = fw_pool.tile([BL, 1], F32, tag="amin")
        nc.vector.memset(amin[:], 0.0)  # "missed" before round 1

        pending = None
        for r in range(n_rounds):
            v = r % NSLOT
            # ---- boundary ----
            # land the previous round's gather (svid update + slot onehot)
            if pending is not None:
                pv, plnd_f = pending
                nc.gpsimd.tensor_copy(out=svid_f[:, pv : pv + 1], in_=plnd_f[:])
                nc.gpsimd.tensor_scalar(
                    out=ohslot[pv][:], in0=iota_h[:], scalar1=plnd_f[:, 0:1],
                    scalar2=0.0, op0=ALU.is_equal, op1=ALU.add,
                )
            # flush victim's A contribution (Pool: mult then add)
            nc.gpsimd.tensor_scalar(
                out=flt[:], in0=ohslot[v][:], scalar1=wslot[:, v : v + 1],
                scalar2=0.0, op0=ALU.mult, op1=ALU.add,
            )
            nc.gpsimd.tensor_add(out=A[:], in0=A[:], in1=flt[:])
            nc.gpsimd.memset(wslot[:, v : v + 1], 0.0)

            # gather idx selection (all on Pool; DVE only runs the steps)
            idx_f = fw_pool.tile([BL, 1], F32, tag="idxf")
            nc.vector.tensor_copy(out=idx_f[:], in_=idx8[:, 0:1])
            mnow = fw_pool.tile([BL, NSLOT], F32, tag="mnow")
            nc.vector.tensor_scalar(
                out=mnow[:], in0=svid_f[:], scalar1=idx_f[:, 0:1], scalar2=0.0,
                op0=ALU.is_equal, op1=ALU.add,
            )
            hitn = fw_pool.tile([BL, 1], F32, tag="hitn")
            nc.vector.tensor_reduce(out=hitn[:], in_=mnow[:], axis=AXL.X, op=ALU.max)
            inval = fw_pool.tile([BL, 1], F32, tag="inval")
            nc.vector.tensor_scalar(
                out=inval[:], in0=svid_f[:, v : v + 1], scalar1=float(INVALID_U),
                scalar2=0.0, op0=ALU.is_equal, op1=ALU.add,
            )
            # gidx = idx + hit*(1-inval)*(svid_v - idx)
            hi = fw_pool.tile([BL, 1], F32, tag="hi")
            nc.vector.tensor_scalar(
                out=hi[:], in0=inval[:], scalar1=-1.0, scalar2=1.0,
                op0=ALU.mult, op1=ALU.add,
            )
            nc.vector.tensor_mul(out=hi[:], in0=hi[:], in1=hitn[:])
            gidx_f = fw_pool.tile([BL, 1], F32, tag="gidxf")
            nc.vector.tensor_sub(out=gidx_f[:], in0=svid_f[:, v : v + 1], in1=idx_f[:])
            nc.vector.tensor_mul(out=gidx_f[:], in0=gidx_f[:], in1=hi[:])
            nc.vector.tensor_add(out=gidx_f[:], in0=gidx_f[:], in1=idx_f[:])
            # landing id: INVALID when hit & victim-was-invalid (duplicate guard)
            dup = fw_pool.tile([BL, 1], F32, tag="dup")
            nc.vector.tensor_mul(out=dup[:], in0=hitn[:], in1=inval[:])
            lnd_f = fw_pool.tile([BL, 1], F32, tag="lndf")
            nc.vector.tensor_scalar(
                out=lnd_f[:], in0=gidx_f[:], scalar1=-1.0, scalar2=float(INVALID_U),
                op0=ALU.mult, op1=ALU.add,
            )
            nc.vector.tensor_mul(out=lnd_f[:], in0=lnd_f[:], in1=dup[:])
            nc.vector.tensor_add(out=lnd_f[:], in0=lnd_f[:], in1=gidx_f[:])
            gidx_u = fw_pool.tile([BL, 1], U32, tag="gidxu")
            nc.vector.tensor_copy(out=gidx_u[:], in_=gidx_f[:])
            # invalidate victim for this round (after gidx computed)
            nc.vector.memset(svid_f[:, v : v + 1], float(INVALID_U))

            idxg = fw_pool.tile([BL, 1], U32, tag="idxg")
            nc.gpsimd.tensor_add(out=idxg[:], in0=gidx_u[:], in1=rowbase[:])
            nc.gpsimd.indirect_dma_start(
                out=cache[v][:],
                out_offset=None,
                in_=mt_dram[:],
                in_offset=bass.IndirectOffsetOnAxis(ap=idxg[:, 0:1], axis=0),
            )
            pending = (v, lnd_f)

            if r == 0:
                continue

            # ---- G steps from cache (victim slot v excluded) ----
            slots = [s for s in range(NSLOT) if s != v]
            for g in range(FW_G):
                if step_no >= FW_T:
                    break
                step_no += 1
                # wneg = -(t+1) ; a_s = (svid_s == idx) * wneg
                wneg = fw_pool.tile([BL, 1], F32, tag="wneg")
                nc.vector.tensor_scalar(
                    out=wneg[:], in0=tcnt[:], scalar1=1.0, scalar2=-1.0,
                    op0=ALU.add, op1=ALU.mult,
                )
                idx_fs = fw_pool.tile([BL, 1], F32, tag="idxfs")
                nc.vector.tensor_copy(out=idx_fs[:], in_=idx8[:, 0:1])
                a = fw_pool.tile([BL, NSLOT], F32, tag="a")
                nc.vector.tensor_scalar(
                    out=a[:], in0=svid_f[:], scalar1=idx_fs[:, 0:1],
                    scalar2=wneg[:, 0:1], op0=ALU.is_equal, op1=ALU.mult,
                )
                amin = fw_pool.tile([BL, 1], F32, tag="amin")
                nc.vector.tensor_reduce(out=amin[:], in_=a[:], axis=AXL.X, op=ALU.min)
                # Pool bookkeeping: live, t += live, wslot -= a
                live = fw_pool.tile([BL, 1], F32, tag="live")
                nc.gpsimd.tensor_scalar(
                    out=live[:], in0=amin[:], scalar1=-0.5, scalar2=0.0,
                    op0=ALU.is_lt, op1=ALU.add,
                )
                nc.gpsimd.tensor_add(out=tcnt[:], in0=tcnt[:], in1=live[:])
                nc.gpsimd.tensor_sub(out=wslot[:], in0=wslot[:], in1=a[:])
                # u update over the two non-victim slots
                if step_no == 1:
                    s0, s1 = slots
                    nc.vector.tensor_scalar(
                        out=u[:], in0=cache[s0][:], scalar1=a[:, s0 : s0 + 1],
                        scalar2=0.0, op0=ALU.mult, op1=ALU.add,
                    )
                    nc.vector.scalar_tensor_tensor(
                        out=u[:], in0=cache[s1][:], scalar=a[:, s1 : s1 + 1],
                        in1=u[:], op0=ALU.mult, op1=ALU.add,
                    )
                else:
                    for s in slots:
                        nc.vector.scalar_tensor_tensor(
                            out=u[:], in0=cache[s][:], scalar=a[:, s : s + 1],
                            in1=u[:], op0=ALU.mult, op1=ALU.add,
                        )
                vals8 = fw_pool.tile([BL, 8], F16, tag="vals8")
                idx8 = fw_pool.tile([BL, 8], U32, tag="idx8")
                nc.vector.max(out=vals8[:], in_=u[:])
                nc.vector.max_index(out=idx8[:], in_max=vals8[:], in_values=u[:])

        # ---- final flush of all slots + val ----
        for s in range(NSLOT):
            nc.vector.scalar_tensor_tensor(
                out=A[:], in0=ohslot[s][:], scalar=wslot[:, s : s + 1],
                in1=A[:], op0=ALU.mult, op1=ALU.add,
            )
        junk = singles.tile([BL, K], F32)
        val_sb = singles.tile([BL, 1], F32)
        nc.vector.tensor_mul(out=junk[:], in0=A[:], in1=u[:])
        nc.vector.reduce_sum(out=val_sb[:], in_=junk[:], axis=AXL.X)
        nc.sync.dma_start(out=val_o[:, :], in_=val_sb[:])
        nc.sync.dma_start(out=eps_o[:, :], in_=eps_row[:])
        nc.sync.dma_start(out=fmin_o[:, :], in_=fpack[:, 1:2])
        nc.sync.dma_start(out=t_o[:, :], in_=tcnt[:])


_NC = None


def _get_nc():
    global _NC
    if _NC is None:
        nc = bacc.Bacc(
            "TRN2",
            target_bir_lowering=False,
            debug=False,
            enable_asserts=False,
            num_devices=NCORES,
        )
        C_l = nc.dram_tensor("C_l", (BL, K, K), F32, kind="ExternalInput").ap()
        scores_l = nc.dram_tensor("scores_l", (BL, K), F32, kind="ExternalInput").ap()
        val_o = nc.dram_tensor("val_o", (BL, 1), F32, kind="ExternalOutput").ap()
        eps_o = nc.dram_tensor("eps_o", (1, BL), F32, kind="ExternalOutput").ap()
        fmin_o = nc.dram_tensor("fmin_o", (BL, 1), F32, kind="ExternalOutput").ap()
        t_o = nc.dram_tensor("t_o", (BL, 1), F32, kind="ExternalOutput").ap()
        with tile.TileContext(nc) as tc:
            _kernel_body(tc, C_l, scores_l, val_o, eps_o, fmin_o, t_o)
        nc.compile()
        _NC = nc
    return _NC


def _finish(results, scores, targets):
    vals = np.concatenate([r["val_o"][:, 0] for r in results]).astype(np.float64)
    eps = np.concatenate([r["eps_o"][0, :] for r in results]).astype(np.float64)
    fmin = np.concatenate([r["fmin_o"][:, 0] for r in results]).astype(np.float64)
    t = np.concatenate([r["t_o"][:, 0] for r in results]).astype(np.float64)
    W = np.maximum(t * (t + 1.0) / 2.0, 1.0)
    val = np.maximum(-vals / (W * W), 1e-300)
    f_y = scores[np.arange(B), targets].astype(np.float64)
    loss = -eps * (np.log(val) - EXP_SHIFT) + 2.0 * fmin - f_y
    return np.float32(loss.mean())


def _run(inputs, **spmd_kwargs):
    scores = np.ascontiguousarray(np.asarray(inputs["scores"], dtype=np.float32))
    targets = np.asarray(inputs["targets"]).astype(np.int64)
    C = np.asarray(inputs["C"], dtype=np.float32)
    nc = _get_nc()
    in_maps = []
    for c in range(NCORES):
        sl = slice(c * BL, (c + 1) * BL)
        in_maps.append(
            {
                "C_l": np.ascontiguousarray(C[sl]),
                "scores_l": np.ascontiguousarray(scores[sl]),
            }
        )
    res = run_bass_kernel_spmd(nc, in_maps, core_ids=list(range(NCORES)), **spmd_kwargs)
    return _finish(res.results, scores, targets), res


def kernel(**inputs) -> np.ndarray:
    out, _ = _run(inputs)
    return out


# revision 17
# speedup vs baseline: 1.1437x; 1.0922x over previous
"""Trainium2 Bass kernel for nn_CACISLoss_78761110274122.

Strategy (pure data parallel, 8 cores x 64 batches):
  Build (per batch b):  eps_b = offdiag_mean(C_b);  T_ij = f_i + f_j + C_ij
                M'_ij = exp((Tlow_b - T_ij)/eps_b + ES)  (Tlow_b = 2*min f_i)
                stored transposed (M'^T rows = M' columns) in DRAM f16.
  Frank-Wolfe via cached multi-step rounds: the FW trajectory revisits a
  tiny vertex set per batch (<=8 distinct over 50 iters), so instead of one
  indirect-DMA column gather per iteration (~2.7us fixed SWDGE+DMA latency),
  keep a 4-slot SBUF column cache per batch. Each round: one bulk gather
  refills a rotating victim slot (missed vertices, or a re-fetch to keep the
  slot warm); G steps then run entirely from SBUF:
      idx=argmax(u); hit slots matched by vertex id; per-batch weight
      w=(t+1)*live; u -= w*col_slot; per-slot weight accum; t += live.
  Batches whose vertex is uncached freeze (u unchanged) until their column
  lands, then resume; per-batch step counters t_b make the lockstep exact.
  alpha is reconstructed from per-slot weight accumulators flushed on slot
  eviction:  A = sum_s Wslot_s * onehot(svid_s).
  Final:  val_b = -(A.u)/W_b^2 = alpha^T M' alpha,  W_b = t_b(t_b+1)/2.
  Loss finish on host in f64: -eps*(log val - ES) + 2*fmin - f_y.
"""

import os
from contextlib import ExitStack

import numpy as np

import concourse.bacc as bacc
import concourse.bass as bass
import concourse.tile as tile
from concourse import mybir
from concourse.bass_utils import run_bass_kernel_spmd
from concourse.masks import make_identity

B, K = 512, 256
NCORES = 8
BL = B // NCORES  # 64 batches per core
N_STEPS = 50
F32 = mybir.dt.float32
F16 = mybir.dt.float16
U32 = mybir.dt.uint32
EXP_SHIFT = 1.0
NSLOT = 3
FW_G = int(os.environ.get("KM_G", "3"))  # steps per round
FW_T = int(os.environ.get("KM_T", "48"))  # total step slots
INVALID_U = 65535  # slot-id sentinel, never equals a vertex id (< 256)
ALU = mybir.AluOpType
AFT = mybir.ActivationFunctionType
AXL = mybir.AxisListType


def _build_mt(tc, ctx, C_l, scores_l, singles, eps_row, fpack, mt_dram, u):
    """Build M'^T into mt_dram (f16) + row sums -> u (f16), eps/fmin tiles.

    Identical structure to the tuned baseline build: group-pipelined
    C load -> T build (+f_i) -> transpose -> exp(scale,bias) -> row sums.
    """
    nc = tc.nc
    ct_pool = ctx.enter_context(tc.tile_pool(name="ct", bufs=6))
    mt_pool = ctx.enter_context(tc.tile_pool(name="mt", bufs=3))
    eps_pool = ctx.enter_context(tc.tile_pool(name="eps", bufs=2))
    ps_small = ctx.enter_context(tc.tile_pool(name="psS", bufs=2, space="PSUM"))
    ps_tt = ctx.enter_context(tc.tile_pool(name="psTT", bufs=4, space="PSUM"))
    ps_r0 = ctx.enter_context(tc.tile_pool(name="psR0", bufs=1, space="PSUM"))

    ident = singles.tile([128, 128], F32)
    make_identity(nc, ident[:])
    ones_col = singles.tile([128, 1], F32)
    nc.vector.memset(ones_col[:], 1.0)
    ones_col_h = singles.tile([128, 1], F16)
    nc.vector.memset(ones_col_h[:], 1.0)
    ones_row = singles.tile([1, 128], F32)
    nc.vector.memset(ones_row[:], 1.0)

    # ---- scores -> f = scores/2, reductions, row/col layouts ----
    scores_sb = singles.tile([BL, K], F32)
    nc.sync.dma_start(out=scores_sb[:], in_=scores_l[:, :])
    fhalf = singles.tile([BL, K], F32)
    nc.vector.tensor_scalar_mul(fhalf[:], scores_sb[:], 0.5)
    nc.vector.reduce_sum(out=fpack[:, 0:1], in_=fhalf[:], axis=AXL.X)
    nc.vector.tensor_reduce(out=fpack[:, 1:2], in_=fhalf[:], axis=AXL.X, op=ALU.min)
    fT_ps = ps_small.tile([128, 2 * BL], F32, tag="small")
    for ib in range(2):
        nc.tensor.transpose(
            out=fT_ps[:, ib * BL : (ib + 1) * BL],
            in_=fhalf[:, ib * 128 : (ib + 1) * 128],
            identity=ident[0:BL, 0:BL],
        )
    fT_sb = singles.tile([128, 2 * BL], F32)
    nc.vector.tensor_copy(out=fT_sb[:], in_=fT_ps[:])

    GRP = 8
    NG = BL // GRP
    collector = singles.tile([128, 4 * BL], F32)

    fpT_ps = ps_small.tile([1, 2 * BL], F32, tag="small")
    for c in range(2):
        nc.tensor.transpose(
            out=fpT_ps[:, c * BL : (c + 1) * BL],
            in_=fpack[:, c : c + 1],
            identity=ident[0:BL, 0:BL],
        )
    frows = singles.tile([1, 2 * BL], F32)
    nc.vector.tensor_copy(out=frows[:], in_=fpT_ps[:])

    r0c = ps_r0.tile([128, K], F32)
    scb = singles.tile([128, 16 * NG], F32)
    biasv = singles.tile([128, 2 * BL], F32)
    coll2 = collector[:].rearrange("p (s c) -> p s c", s=2)

    # group schedule: batches [b0, b0+gsz); last 8-block split into 4+4 to
    # shorten the end-of-build pipeline drain. The eps chain runs per 8-block
    # once that block's accumulations are complete.
    sched = [(g * 8, 8) for g in range(NG - 1)] + [((NG - 1) * 8, 4), ((NG - 1) * 8 + 4, 4)]

    def emit_eps_block(g):
        gs = slice(g * 8, (g + 1) * 8)
        colsum_ps = ps_small.tile([1, 32], F32, tag="small", name="colsum_ps")
        nc.tensor.matmul(
            out=colsum_ps[:],
            lhsT=ones_col[:],
            rhs=coll2[:, :, g * 16 : g * 16 + 16],
            start=True,
            stop=True,
        )
        srow = eps_pool.tile([1, 32], F32, tag="srow")
        nc.vector.tensor_copy(out=srow[:], in_=colsum_ps[:])
        sc = eps_pool.tile([1, 8], F32, tag="sc")
        nc.vector.tensor_add(out=sc[:], in0=srow[0:1, 0:8], in1=srow[0:1, 8:16])
        nc.vector.scalar_tensor_tensor(
            out=sc[:], in0=frows[0:1, gs], scalar=-1.0 * K, in1=sc[:],
            op0=ALU.mult, op1=ALU.add,
        )
        tr = eps_pool.tile([1, 8], F32, tag="tr")
        nc.vector.tensor_add(out=tr[:], in0=srow[0:1, 16:24], in1=srow[0:1, 24:32])
        nc.vector.tensor_sub(out=sc[:], in0=sc[:], in1=tr[:])
        nc.vector.tensor_scalar(
            out=eps_row[0:1, gs], in0=sc[:], scalar1=1.0 / (K * K - K),
            scalar2=1e-8, op0=ALU.mult, op1=ALU.max,
        )
        rec = eps_pool.tile([1, 8], F32, tag="rec")
        nc.vector.reciprocal(out=rec[:], in_=eps_row[0:1, gs])
        sr = eps_pool.tile([1, 8], F32, tag="sr")
        nc.vector.tensor_scalar_mul(sr[:], rec[:], -1.0)
        br = eps_pool.tile([1, 8], F32, tag="br")
        nc.vector.scalar_tensor_tensor(
            out=br[:], in0=frows[0:1, BL + g * 8 : BL + (g + 1) * 8],
            scalar=-2.0, in1=sr[:], op0=ALU.mult, op1=ALU.mult,
        )
        nc.vector.tensor_scalar_add(br[:], br[:], EXP_SHIFT)
        scb_ps = ps_small.tile([128, 16], F32, tag="small", name="scb_ps")
        nc.tensor.matmul(
            out=scb_ps[:, 0:8], lhsT=ones_row[:, :], rhs=sr[:], start=True, stop=True
        )
        nc.tensor.matmul(
            out=scb_ps[:, 8:16], lhsT=ones_row[:, :], rhs=br[:], start=True, stop=True
        )
        nc.vector.tensor_copy(out=scb[:, g * 16 : (g + 1) * 16], in_=scb_ps[:])
        for jb in range(2):
            sl = slice(jb * BL + g * 8, jb * BL + (g + 1) * 8)
            nc.vector.tensor_mul(
                out=biasv[:, sl], in0=fT_sb[:, sl], in1=scb[:, g * 16 : g * 16 + 8]
            )
            nc.vector.tensor_add(
                out=biasv[:, sl], in0=biasv[:, sl],
                in1=scb[:, g * 16 + 8 : g * 16 + 16],
            )

    def emit_B(b0, gsz, ct):
        mt_sb = mt_pool.tile([128, 2 * gsz, K], F16, tag="mt", name="mt_sb")
        for b2 in range(gsz):
            b = b0 + b2
            g = b // 8
            tt_ps = ps_tt.tile([128, 2, K], F32, tag="tt", name="tt_ps")
            for jb in range(2):
                for ib in range(2):
                    nc.tensor.transpose(
                        out=tt_ps[:, jb, ib * 128 : (ib + 1) * 128],
                        in_=ct[:, b2 * 2 + ib, jb * 128 : (jb + 1) * 128],
                        identity=ident[:],
                    )
            for jb in range(2):
                m = b2 * 2 + jb
                nc.scalar.activation(
                    out=mt_sb[:, m, :],
                    in_=tt_ps[:, jb, :],
                    func=AFT.Exp,
                    bias=biasv[:, jb * BL + b : jb * BL + b + 1],
                    scale=scb[:, g * 16 + (b % 8) : g * 16 + (b % 8) + 1],
                )
                for ib in range(2):
                    col = jb * 128 + ib * BL + b
                    nc.tensor.matmul(
                        out=r0c[:, col : col + 1],
                        lhsT=mt_sb[:, m, ib * 128 : (ib + 1) * 128],
                        rhs=ones_col_h[:],
                        start=True,
                        stop=True,
                    )
        dst_ap = bass.AP(
            tensor=mt_dram.tensor,
            offset=b0 * K * K,
            ap=[[K, 128], [128 * K, 2 * gsz], [1, K]],
        )
        nc.sync.dma_start(out=dst_ap, in_=mt_sb[:])

    pending = []
    for b0, gsz in sched:
        ct = ct_pool.tile([128, 2 * gsz, K], F32, tag="ct", name="ct")
        src_ap = bass.AP(
            tensor=C_l.tensor,
            offset=b0 * K * K,
            ap=[[K, 128], [128 * K, 2 * gsz], [1, K]],
        )
        nc.sync.dma_start(out=ct[:], in_=src_ap)
        for ih in range(2):
            diag_ap = bass.AP(
                tensor=C_l.tensor,
                offset=b0 * K * K + ih * (K + 1) * 128,
                ap=[[K + 1, 128], [K * K, gsz]],
            )
            c0 = 2 * BL + (b0 // 8) * 16 + ih * 8 + (b0 % 8)
            nc.scalar.dma_start(out=collector[:, c0 : c0 + gsz], in_=diag_ap)
        for b2 in range(gsz):
            b = b0 + b2
            for ib in range(2):
                c0 = (b // 8) * 16 + ib * 8 + (b % 8)
                nc.vector.tensor_scalar(
                    out=ct[:, b2 * 2 + ib, :],
                    in0=ct[:, b2 * 2 + ib, :],
                    scalar1=fT_sb[:, ib * BL + b : ib * BL + b + 1],
                    scalar2=0.0,
                    op0=ALU.add,
                    op1=ALU.add,
                    accum_out=collector[:, c0 : c0 + 1],
                )
        pending.append((b0, gsz, ct))
        if (b0 + gsz) % 8 == 0:
            emit_eps_block((b0 + gsz) // 8 - 1)
            for pb0, pgsz, pct in pending:
                emit_B(pb0, pgsz, pct)
            pending = []

    # rowsums -> u0 = -rowsum (f16)
    r0s = singles.tile([128, 128], F32)
    nc.vector.tensor_copy(out=r0s[:], in_=r0c[:, 0:128])
    nc.vector.tensor_add(out=r0s[:], in0=r0s[:], in1=r0c[:, 128:K])
    r0T_ps = ps_small.tile([128, 128], F32, tag="small")
    nc.tensor.transpose(out=r0T_ps[:], in_=r0s[:], identity=ident[:])
    nc.vector.tensor_scalar_mul(u[:, 0:128], r0T_ps[0:BL, :], -1.0)
    nc.vector.tensor_scalar_mul(u[:, 128:K], r0T_ps[BL : 2 * BL, :], -1.0)


def _kernel_body(tc, C_l, scores_l, val_o, eps_o, fmin_o, t_o):
    nc = tc.nc
    with ExitStack() as ctx:
        singles = ctx.enter_context(tc.tile_pool(name="singles", bufs=1))
        fw_pool = ctx.enter_context(tc.tile_pool(name="fw", bufs=3))
        dram = ctx.enter_context(tc.tile_pool(name="dram", bufs=1, space="DRAM"))

        mt_dram = dram.tile([BL * K, K], F16)
        eps_row = singles.tile([1, BL], F32)
        fpack = singles.tile([BL, 2], F32)
        u = singles.tile([BL, K], F16)

        _build_mt(tc, ctx, C_l, scores_l, singles, eps_row, fpack, mt_dram, u)

        # ---- FW state ----
        rowbase = singles.tile([BL, 1], U32)
        nc.gpsimd.iota(rowbase[:], pattern=[[0, 1]], base=0, channel_multiplier=K)
        iota_u = singles.tile([BL, K], U32)
        nc.gpsimd.iota(iota_u[:], pattern=[[1, K]], base=0, channel_multiplier=0)
        iota_h = singles.tile([BL, K], F16)
        nc.vector.tensor_copy(out=iota_h[:], in_=iota_u[:])

        cache = [singles.tile([BL, K], F16, name=f"cache{s}") for s in range(NSLOT)]
        ohslot = [singles.tile([BL, K], F16, name=f"ohslot{s}") for s in range(NSLOT)]
        for s in range(NSLOT):
            nc.vector.memset(cache[s][:], 0.0)
            nc.vector.memset(ohslot[s][:], 0.0)
        svid_f = singles.tile([BL, NSLOT], F32)
        nc.vector.memset(svid_f[:], float(INVALID_U))
        wslot = singles.tile([BL, NSLOT], F32)
        nc.vector.memset(wslot[:], 0.0)
        A = singles.tile([BL, K], F32)
        nc.vector.memset(A[:], 0.0)
        tcnt = singles.tile([BL, 1], F32)
        nc.gpsimd.memset(tcnt[:], 0.0)
        flt = singles.tile([BL, K], F32)  # flush scratch (Pool)

        n_rounds = (FW_T + FW_G - 1) // FW_G + 1
        step_no = 0

        # initial argmax of u0
        vals8 = fw_pool.tile([BL, 8], F16, tag="vals8")
        idx8 = fw_pool.tile([BL, 8], U32, tag="idx8")
        nc.vector.max(out=vals8[:], in_=u[:])
        nc.vector.max_index(out=idx8[:], in_max=vals8[:], in_values=u[:])
        amin = fw_pool.tile([BL, 1], F32, tag="amin")
        nc.vector.memset(amin[:], 0.0)  # "missed" before round 1

        pending = None
        for r in range(n_rounds):
            v = r % NSLOT
            # ---- boundary ----
            # land the previous round's gather (svid update + slot onehot)
            if pending is not None:
                pv, plnd_f = pending
                nc.gpsimd.tensor_copy(out=svid_f[:, pv : pv + 1], in_=plnd_f[:])
                nc.vector.tensor_scalar(
                    out=ohslot[pv][:], in0=iota_h[:], scalar1=plnd_f[:, 0:1],
                    scalar2=0.0, op0=ALU.is_equal, op1=ALU.add,
                )
            # flush victim's A contribution (Pool: mult then add)
            nc.gpsimd.tensor_scalar(
                out=flt[:], in0=ohslot[v][:], scalar1=wslot[:, v : v + 1],
                scalar2=0.0, op0=ALU.mult, op1=ALU.add,
            )
            nc.gpsimd.tensor_add(out=A[:], in0=A[:], in1=flt[:])
            nc.gpsimd.memset(wslot[:, v : v + 1], 0.0)

            # gather idx selection (all on Pool; DVE only runs the steps)
            idx_f = fw_pool.tile([BL, 1], F32, tag="idxf")
            nc.vector.tensor_copy(out=idx_f[:], in_=idx8[:, 0:1])
            mnow = fw_pool.tile([BL, NSLOT], F32, tag="mnow")
            nc.vector.tensor_scalar(
                out=mnow[:], in0=svid_f[:], scalar1=idx_f[:, 0:1], scalar2=0.0,
                op0=ALU.is_equal, op1=ALU.add,
            )
            hitn = fw_pool.tile([BL, 1], F32, tag="hitn")
            nc.vector.tensor_reduce(out=hitn[:], in_=mnow[:], axis=AXL.X, op=ALU.max)
            inval = fw_pool.tile([BL, 1], F32, tag="inval")
            nc.vector.tensor_scalar(
                out=inval[:], in0=svid_f[:, v : v + 1], scalar1=float(INVALID_U),
                scalar2=0.0, op0=ALU.is_equal, op1=ALU.add,
            )
            # gidx = idx + hit*(1-inval)*(svid_v - idx)
            hi = fw_pool.tile([BL, 1], F32, tag="hi")
            nc.vector.tensor_scalar(
                out=hi[:], in0=inval[:], scalar1=-1.0, scalar2=1.0,
                op0=ALU.mult, op1=ALU.add,
            )
            nc.vector.tensor_mul(out=hi[:], in0=hi[:], in1=hitn[:])
            gidx_f = fw_pool.tile([BL, 1], F32, tag="gidxf")
            nc.vector.tensor_sub(out=gidx_f[:], in0=svid_f[:, v : v + 1], in1=idx_f[:])
            nc.vector.tensor_mul(out=gidx_f[:], in0=gidx_f[:], in1=hi[:])
            nc.vector.tensor_add(out=gidx_f[:], in0=gidx_f[:], in1=idx_f[:])
            # landing id: INVALID when hit & victim-was-invalid (duplicate guard)
            dup = fw_pool.tile([BL, 1], F32, tag="dup")
            nc.vector.tensor_mul(out=dup[:], in0=hitn[:], in1=inval[:])
            lnd_f = fw_pool.tile([BL, 1], F32, tag="lndf")
            nc.vector.tensor_scalar(
                out=lnd_f[:], in0=gidx_f[:], scalar1=-1.0, scalar2=float(INVALID_U),
                op0=ALU.mult, op1=ALU.add,
            )
            nc.vector.tensor_mul(out=lnd_f[:], in0=lnd_f[:], in1=dup[:])
            nc.vector.tensor_add(out=lnd_f[:], in0=lnd_f[:], in1=gidx_f[:])
            gidx_u = fw_pool.tile([BL, 1], U32, tag="gidxu")
            nc.vector.tensor_copy(out=gidx_u[:], in_=gidx_f[:])
            # invalidate victim for this round (after gidx computed)
            nc.vector.memset(svid_f[:, v : v + 1], float(INVALID_U))

            idxg = fw_pool.tile([BL, 1], U32, tag="idxg")
            nc.gpsimd.tensor_add(out=idxg[:], in0=gidx_u[:], in1=rowbase[:])
            nc.gpsimd.indirect_dma_start(
                out=cache[v][:],
                out_offset=None,
                in_=mt_dram[:],
                in_offset=bass.IndirectOffsetOnAxis(ap=idxg[:, 0:1], axis=0),
            )
            pending = (v, lnd_f)

            if r == 0:
                continue

            # ---- G steps from cache (victim slot v excluded) ----
            slots = [s for s in range(NSLOT) if s != v]
            for g in range(FW_G):
                if step_no >= FW_T:
                    break
                step_no += 1
                # wneg = -(t+1) ; a_s = (svid_s == idx) * wneg
                wneg = fw_pool.tile([BL, 1], F32, tag="wneg")
                nc.vector.tensor_scalar(
                    out=wneg[:], in0=tcnt[:], scalar1=1.0, scalar2=-1.0,
                    op0=ALU.add, op1=ALU.mult,
                )
                idx_fs = fw_pool.tile([BL, 1], F32, tag="idxfs")
                nc.vector.tensor_copy(out=idx_fs[:], in_=idx8[:, 0:1])
                a = fw_pool.tile([BL, NSLOT], F32, tag="a")
                nc.vector.tensor_scalar(
                    out=a[:], in0=svid_f[:], scalar1=idx_fs[:, 0:1],
                    scalar2=wneg[:, 0:1], op0=ALU.is_equal, op1=ALU.mult,
                )
                amin = fw_pool.tile([BL, 1], F32, tag="amin")
                nc.vector.tensor_reduce(out=amin[:], in_=a[:], axis=AXL.X, op=ALU.min)
                # Pool bookkeeping: live, t += live, wslot -= a
                live = fw_pool.tile([BL, 1], F32, tag="live")
                nc.gpsimd.tensor_scalar(
                    out=live[:], in0=amin[:], scalar1=-0.5, scalar2=0.0,
                    op0=ALU.is_lt, op1=ALU.add,
                )
                nc.gpsimd.tensor_add(out=tcnt[:], in0=tcnt[:], in1=live[:])
                nc.gpsimd.tensor_sub(out=wslot[:], in0=wslot[:], in1=a[:])
                # u update over the two non-victim slots
                if step_no == 1:
                    s0, s1 = slots
                    nc.vector.tensor_scalar(
                        out=u[:], in0=cache[s0][:], scalar1=a[:, s0 : s0 + 1],
                        scalar2=0.0, op0=ALU.mult, op1=ALU.add,
                    )
                    nc.vector.scalar_tensor_tensor(
                        out=u[:], in0=cache[s1][:], scalar=a[:, s1 : s1 + 1],
                        in1=u[:], op0=ALU.mult, op1=ALU.add,
                    )
                else:
                    for s in slots:
                        nc.vector.scalar_tensor_tensor(
                            out=u[:], in0=cache[s][:], scalar=a[:, s : s + 1],
                            in1=u[:], op0=ALU.mult, op1=ALU.add,
                        )
                vals8 = fw_pool.tile([BL, 8], F16, tag="vals8")
                idx8 = fw_pool.tile([BL, 8], U32, tag="idx8")
                nc.vector.max(out=vals8[:], in_=u[:])
                nc.vector.max_index(out=idx8[:], in_max=vals8[:], in_values=u[:])

        # ---- final flush of all slots + val ----
        for s in range(NSLOT):
            nc.vector.scalar_tensor_tensor(
                out=A[:], in0=ohslot[s][:], scalar=wslot[:, s : s + 1],
                in1=A[:], op0=ALU.mult, op1=ALU.add,
            )
        junk = singles.tile([BL, K], F32)
        val_sb = singles.tile([BL, 1], F32)
        nc.vector.tensor_mul(out=junk[:], in0=A[:], in1=u[:])
        nc.vector.reduce_sum(out=val_sb[:], in_=junk[:], axis=AXL.X)
        nc.sync.dma_start(out=val_o[:, :], in_=val_sb[:])
        nc.sync.dma_start(out=eps_o[:, :], in_=eps_row[:])
        nc.sync.dma_start(out=fmin_o[:, :], in_=fpack[:, 1:2])
        nc.sync.dma_start(out=t_o[:, :], in_=tcnt[:])


_NC = None


def _get_nc():
    global _NC
    if _NC is None:
        nc = bacc.Bacc(
            "TRN2",
            target_bir_lowering=False,
            debug=False,
            enable_asserts=False,
            num_devices=NCORES,
        )
        C_l = nc.dram_tensor("C_l", (BL, K, K), F32, kind="ExternalInput").ap()
        scores_l = nc.dram_tensor("scores_l", (BL, K), F32, kind="ExternalInput").ap()
        val_o = nc.dram_tensor("val_o", (BL, 1), F32, kind="ExternalOutput").ap()
        eps_o = nc.dram_tensor("eps_o", (1, BL), F32, kind="ExternalOutput").ap()
        fmin_o = nc.dram_tensor("fmin_o", (BL, 1), F32, kind="ExternalOutput").ap()
        t_o = nc.dram_tensor("t_o", (BL, 1), F32, kind="ExternalOutput").ap()
        with tile.TileContext(nc) as tc:
            _kernel_body(tc, C_l, scores_l, val_o, eps_o, fmin_o, t_o)
        nc.compile()
        _NC = nc
    return _NC


def _finish(results, scores, targets):
    vals = np.concatenate([r["val_o"][:, 0] for r in results]).astype(np.float64)
    eps = np.concatenate([r["eps_o"][0, :] for r in results]).astype(np.float64)
    fmin = np.concatenate([r["fmin_o"][:, 0] for r in results]).astype(np.float64)
    t = np.concatenate([r["t_o"][:, 0] for r in results]).astype(np.float64)
    W = np.maximum(t * (t + 1.0) / 2.0, 1.0)
    val = np.maximum(-vals / (W * W), 1e-300)
    f_y = scores[np.arange(B), targets].astype(np.float64)
    loss = -eps * (np.log(val) - EXP_SHIFT) + 2.0 * fmin - f_y
    return np.float32(loss.mean())


def _run(inputs, **spmd_kwargs):
    scores = np.ascontiguousarray(np.asarray(inputs["scores"], dtype=np.float32))
    targets = np.asarray(inputs["targets"]).astype(np.int64)
    C = np.asarray(inputs["C"], dtype=np.float32)
    nc = _get_nc()
    in_maps = []
    for c in range(NCORES):
        sl = slice(c * BL, (c + 1) * BL)
        in_maps.append(
            {
                "C_l": np.ascontiguousarray(C[sl]),
                "scores_l": np.ascontiguousarray(scores[sl]),
            }
        )
    res = run_bass_kernel_spmd(nc, in_maps, core_ids=list(range(NCORES)), **spmd_kwargs)
    return _finish(res.results, scores, targets), res


def kernel(**inputs) -> np.ndarray:
    out, _ = _run(inputs)
    return out
